# revision 14
# baseline (speedup 1.0000x reference)
"""Trainium2 8-core Bass kernel for CogVideoX attention + conv1d block.

Sharding: heads (tensor-parallel, 30 padded to 32 -> 4 heads/core) for
QKV/attention; out-proj computed as per-core channel partials combined with a
chunked ReduceScatter along the sequence; conv1d branch: conv1 replicated,
conv2 sharded by output channel so it folds into the same partial.

All matmuls in bf16 (fp32 PSUM accumulation). Interleaved RoPE is turned into
a rotate-half form by permuting Q/K weight rows host-side. QKV bias is folded
into the projection via an ones-row in hsT. Softmax denominators come from an
ones-column appended to V. LayerNorm q/k weights are identity (per the module
spec) so only (x-mu)*rsqrt(var+eps) is applied.
"""
import numpy as np
import ml_dtypes

from concourse import bacc, tile, mybir
from concourse import bass_utils

BF16 = mybir.dt.bfloat16
F32 = mybir.dt.float32
BF = ml_dtypes.bfloat16

B, F, H, W = 1, 8, 16, 16
DIM, HEADS, HD, RANK, T = 1920, 30, 64, 128, 226
SV = F * H * W            # 2048
S = T + SV                # 2274
SP = 2304                 # padded S = 18*128
NST = SP // 128           # 18 s-tiles
DP = 2048                 # padded contraction dim (1920 + ones row + zeros)
NKT = DP // 128           # 16 k-tiles
NKC = DIM // 128          # 15 k-tiles for conv (no ones row)
HP = 32                   # padded heads
NCORE = 8
HPC = HP // NCORE         # 4 heads/core
CH = HPC * HD             # 256 channels/core
VCH = HPC * (HD + 1)      # 260 v channels (with ones cols)
OROWS = DIM // NCORE      # 240 output rows/core after ReduceScatter
EPS = 1e-6
QT_SIZES = [512, 512, 512, 512, 256]  # 5 q-tiles covering SP
HW2 = H * W               # 256

PERM = np.concatenate([np.arange(0, 64, 2), np.arange(1, 64, 2)])


def _host_prep(inputs):
    hs = np.asarray(inputs["hidden_states"], np.float32)[0]
    ehs = np.asarray(inputs["encoder_hidden_states"], np.float32)[0]
    cos = np.asarray(inputs["cos"], np.float32)
    sin = np.asarray(inputs["sin"], np.float32)

    hs_cat = np.concatenate([ehs, hs], 0)
    hsT = np.zeros((DP, SP), np.float32)
    hsT[:DIM, :S] = hs_cat.T
    hsT[DIM, :S] = 1.0

    def pad_rows(w, b, deint):
        wp = np.zeros((HP * HD, DIM), np.float32)
        bp = np.zeros((HP * HD,), np.float32)
        wp[:DIM] = np.asarray(w, np.float32)
        bp[:DIM] = np.asarray(b, np.float32)
        if deint:
            wp = wp.reshape(HP, HD, DIM)[:, PERM].reshape(HP * HD, DIM)
            bp = bp.reshape(HP, HD)[:, PERM].reshape(-1)
        return wp, bp

    wq, bq = pad_rows(inputs["to_q_w"], inputs["to_q_b"], True)
    wk, bk = pad_rows(inputs["to_k_w"], inputs["to_k_b"], True)
    wv, bv = pad_rows(inputs["to_v_w"], inputs["to_v_b"], False)

    cosD = cos[:, PERM]
    sinF = sin[:, PERM].copy()
    sinF[:, :32] *= -1.0
    cosP = np.ones((SP, CH), np.float32)
    sinP = np.zeros((SP, CH), np.float32)
    cosP[T:S] = np.tile(cosD, HPC)
    sinP[T:S] = np.tile(sinF, HPC)

    woT_full = np.zeros((HP * HD, DIM), np.float32)
    woT_full[:DIM] = np.asarray(inputs["to_out_w"], np.float32).T

    w1 = np.asarray(inputs["conv1_w"], np.float32)
    w2 = np.asarray(inputs["conv2_w"], np.float32)
    w1T = np.zeros((DIM, 3 * RANK), np.float32)
    for k in range(3):
        w1T[:, RANK * k:RANK * (k + 1)] = w1[:, :, k].T
    bout = np.asarray(inputs["to_out_b"], np.float32)

    in_maps = []
    for i in range(NCORE):
        hsl = slice(CH * i, CH * (i + 1))
        wqkT = np.zeros((DP, 2 * CH), np.float32)
        wqkT[:DIM, :CH] = wq[hsl].T
        wqkT[DIM, :CH] = bq[hsl]
        wqkT[:DIM, CH:] = wk[hsl].T
        wqkT[DIM, CH:] = bk[hsl]

        wvT = np.zeros((DP, VCH), np.float32)
        for h in range(HPC):
            c0 = 65 * h
            rs = slice(CH * i + HD * h, CH * i + HD * (h + 1))
            wvT[:DIM, c0:c0 + HD] = wv[rs].T
            wvT[DIM, c0:c0 + HD] = bv[rs]
            wvT[DIM, c0 + HD] = 1.0

        w2T = np.zeros((RANK, 3 * CH), np.float32)
        lo, hi = CH * i, min(CH * (i + 1), DIM)
        for k in range(3):
            blk = np.zeros((CH, RANK), np.float32)
            if lo < DIM:
                blk[:hi - lo] = w2[lo:hi, :, k]
            w2T[:, CH * k:CH * (k + 1)] = blk.T

        bslice = bout[OROWS * i:OROWS * (i + 1)].reshape(OROWS, 1).copy()

        in_maps.append({
            "hsT": hsT.astype(BF),
            "wqkT": wqkT.astype(BF),
            "wvT": wvT.astype(BF),
            "woT": woT_full[hsl].astype(BF),
            "w1T": w1T.astype(BF),
            "w2T": w2T.astype(BF),
            "cosP": cosP.astype(BF),
            "sinP": sinP.astype(BF),
            "bout": bslice.astype(np.float32),
        })
    return in_maps


def build_nc(repeat=1):
    nc = bacc.Bacc("TRN2", target_bir_lowering=False, debug=False,
                   num_devices=NCORE)

    hsT_d = nc.dram_tensor("hsT", [DP, SP], BF16, kind="ExternalInput").ap()
    wqkT_d = nc.dram_tensor("wqkT", [DP, 2 * CH], BF16, kind="ExternalInput").ap()
    wvT_d = nc.dram_tensor("wvT", [DP, VCH], BF16, kind="ExternalInput").ap()
    woT_d = nc.dram_tensor("woT", [CH, DIM], BF16, kind="ExternalInput").ap()
    w1T_d = nc.dram_tensor("w1T", [DIM, 3 * RANK], BF16, kind="ExternalInput").ap()
    w2T_d = nc.dram_tensor("w2T", [RANK, 3 * CH], BF16, kind="ExternalInput").ap()
    cosP_d = nc.dram_tensor("cosP", [SP, CH], BF16, kind="ExternalInput").ap()
    sinP_d = nc.dram_tensor("sinP", [SP, CH], BF16, kind="ExternalInput").ap()
    bout_d = nc.dram_tensor("bout", [OROWS, 1], F32, kind="ExternalInput").ap()
    y_d = nc.dram_tensor("yslice", [OROWS, SP], F32, kind="ExternalOutput").ap()

    with tile.TileContext(nc) as tc:
        for _ in range(repeat):
            _body(nc, tc, hsT_d, wqkT_d, wvT_d, woT_d, w1T_d, w2T_d,
                  cosP_d, sinP_d, bout_d, y_d)
    nc.compile()
    return nc


def _body(nc, tc, hsT_d, wqkT_d, wvT_d, woT_d, w1T_d, w2T_d,
          cosP_d, sinP_d, bout_d, y_d):
    from contextlib import ExitStack
    stack = ExitStack()

    def pool(name, bufs, space="SBUF"):
        return stack.enter_context(tc.tile_pool(name=name, bufs=bufs, space=space))

    const = pool("const", 1)
    dram = pool("dram", 1, space="DRAM")

    # phase-A-scoped SBUF pool (wqk/wv weights die after phase A)
    stackA = ExitStack()
    aw_po = stackA.enter_context(tc.tile_pool(name="aw", bufs=1))

    # ---- resident tiles + input DMAs (interleaved so k-tile 0's deps land
    # first and the first QKV matmuls start before the whole 12 MB arrives) --
    hsT_t, wqk_t, wv_t = [], [], []
    for k in range(NKT):
        t = const.tile([128, SP], BF16, name=f"hsT_{k}")
        nc.sync.dma_start(t[:, :], hsT_d[128 * k:128 * (k + 1), :])
        hsT_t.append(t)
        t = aw_po.tile([128, 2 * CH], BF16, name=f"wqk_{k}")
        nc.sync.dma_start(t[:, :], wqkT_d[128 * k:128 * (k + 1), :])
        wqk_t.append(t)
        t = aw_po.tile([128, VCH], BF16, name=f"wv_{k}")
        nc.sync.dma_start(t[:, :], wvT_d[128 * k:128 * (k + 1), :])
        wv_t.append(t)
    wo_t = []
    for m in range(2):
        t = const.tile([128, DIM], BF16, name=f"wo_{m}")
        nc.sync.dma_start(t[:, :], woT_d[128 * m:128 * (m + 1), :])
        wo_t.append(t)
    w1_t = []
    for k in range(NKC):
        t = const.tile([128, 3 * RANK], BF16, name=f"w1_{k}")
        nc.sync.dma_start(t[:, :], w1T_d[128 * k:128 * (k + 1), :])
        w1_t.append(t)
    w2_t = const.tile([128, 3 * CH], BF16, name="w2")
    nc.sync.dma_start(w2_t[:, :], w2T_d[:, :])
    eps_t = const.tile([128, 1], F32, name="eps")
    nc.vector.memset(eps_t[:, :], EPS)
    bout_t = const.tile([128, 2], F32, name="bout")
    nc.sync.dma_start(bout_t[0:128, 0:1], bout_d[0:128, :])
    nc.sync.dma_start(bout_t[0:OROWS - 128, 1:2], bout_d[128:OROWS, :])

    QT = [const.tile([128, SP], BF16, name=f"QT_{j}") for j in range(2)]
    KT = [const.tile([128, SP], BF16, name=f"KT_{j}") for j in range(2)]
    Vt = [const.tile([128, VCH], BF16, name=f"V_{st}") for st in range(NST)]
    attnT = [const.tile([128, SP], BF16, name=f"attnT_{m}") for m in range(2)]
    convT = [const.tile([128, SV], BF16, name=f"convT_{m}") for m in range(2)]
    

    # ---- work pools ----
    # PSUM (8 banks) and SBUF are phase-scoped: phase A pools close before
    # phase B's gelu scratch allocates, which closes before phase C pools.
    qk_ps = stackA.enter_context(tc.tile_pool(name="qk_ps", bufs=2, space="PSUM"))
    v_ps = stackA.enter_context(tc.tile_pool(name="v_ps", bufs=2, space="PSUM"))
    sq_po = stackA.enter_context(tc.tile_pool(name="sq", bufs=2))
    st_po = stackA.enter_context(tc.tile_pool(name="stats", bufs=3))
    strip_po = stackA.enter_context(tc.tile_pool(name="strip", bufs=3))
    rope_po = stackA.enter_context(tc.tile_pool(name="rope", bufs=2))
    cos_po = stackA.enter_context(tc.tile_pool(name="cos", bufs=2))

    AT = mybir.ActivationFunctionType
    OP = mybir.AluOpType
    AX = mybir.AxisListType

    # ================= Phase A: QKV + LN + RoPE + transposes =================
    for st in range(NST):
        s0 = 128 * st
        qk_p = qk_ps.tile([128, 2 * CH], F32, name="qk_p")
        v_p = v_ps.tile([128, VCH], F32, name="v_p")
        for kt in range(NKT):
            lhsT = hsT_t[kt][:, s0:s0 + 128]
            nc.tensor.matmul(qk_p[:, :], lhsT, wqk_t[kt][:, :],
                             start=(kt == 0), stop=(kt == NKT - 1))
            nc.tensor.matmul(v_p[:, :], lhsT, wv_t[kt][:, :],
                             start=(kt == 0), stop=(kt == NKT - 1))
        # V straight to bf16 (bias already folded via ones row)
        nc.scalar.activation(Vt[st][:, :], v_p[:, :], AT.Copy)

        # layernorm stats per 64-col group (8 groups: 4 q heads + 4 k heads)
        sq = sq_po.tile([128, 2 * CH], F32, name="sq")
        nc.scalar.activation(sq[:, :], qk_p[:, :], AT.Square)
        sums = st_po.tile([128, 8], F32, name="sums")
        sqs = st_po.tile([128, 8], F32, name="sqs")
        nc.vector.tensor_reduce(sums[:, :],
                                qk_p[:, :].rearrange("p (g d) -> p g d", g=8),
                                AX.X, OP.add)
        nc.vector.tensor_reduce(sqs[:, :],
                                sq[:, :].rearrange("p (g d) -> p g d", g=8),
                                AX.X, OP.add)
        mu = st_po.tile([128, 8], F32, name="mu")
        nc.vector.tensor_scalar_mul(mu[:, :], sums[:, :], 1.0 / HD)
        msq = st_po.tile([128, 8], F32, name="msq")
        nc.vector.tensor_scalar_mul(msq[:, :], sqs[:, :], 1.0 / HD)
        mu2 = st_po.tile([128, 8], F32, name="mu2")
        nc.vector.tensor_tensor(mu2[:, :], mu[:, :], mu[:, :], OP.mult)
        var = st_po.tile([128, 8], F32, name="var")
        nc.vector.tensor_tensor(var[:, :], msq[:, :], mu2[:, :], OP.subtract)
        std = st_po.tile([128, 8], F32, name="std")
        nc.scalar.activation(std[:, :], var[:, :], AT.Sqrt, bias=eps_t[:, 0:1])
        rstd = st_po.tile([128, 8], F32, name="rstd")
        nc.vector.reciprocal(rstd[:, :], std[:, :])

        strip = strip_po.tile([128, 2 * CH], BF16, name="strip")
        for g in range(8):
            nc.vector.tensor_scalar(strip[:, HD * g:HD * (g + 1)],
                                    qk_p[:, HD * g:HD * (g + 1)],
                                    mu[:, g:g + 1], rstd[:, g:g + 1],
                                    OP.subtract, OP.mult)

        # rope on q half and k half (identity rows cover text/pad)
        cos_t = cos_po.tile([128, CH], BF16, name="cos_t")
        sin_t = cos_po.tile([128, CH], BF16, name="sin_t")
        nc.sync.dma_start(cos_t[:, :], cosP_d[s0:s0 + 128, :])
        nc.sync.dma_start(sin_t[:, :], sinP_d[s0:s0 + 128, :])
        for part in range(2):
            c0 = CH * part
            x3 = strip[:, c0:c0 + CH].rearrange("p (h d) -> p h d", h=HPC)
            rot = rope_po.tile([128, CH], BF16, name="rot")
            rot3 = rot[:, :].rearrange("p (h d) -> p h d", h=HPC)
            sin3 = sin_t[:, :].rearrange("p (h d) -> p h d", h=HPC)
            # rot*sinF: lower half reads upper, upper half reads lower
            nc.vector.tensor_tensor(rot3[:, :, 0:32], x3[:, :, 32:64],
                                    sin3[:, :, 0:32], OP.mult)
            nc.vector.tensor_tensor(rot3[:, :, 32:64], x3[:, :, 0:32],
                                    sin3[:, :, 32:64], OP.mult)
            xc = rope_po.tile([128, CH], BF16, name="xc")
            nc.vector.tensor_tensor(xc[:, :], strip[:, c0:c0 + CH],
                                    cos_t[:, :], OP.mult)
            nc.vector.tensor_tensor(strip[:, c0:c0 + CH], xc[:, :],
                                    rot[:, :], OP.add)

        # transpose into QT/KT via DMA transpose (bf16, 128x128 blocks)
        for j in range(2):
            nc.sync.dma_start(QT[j][:, s0:s0 + 128],
                              strip[:, 128 * j:128 * (j + 1)], transpose=True)
            nc.sync.dma_start(KT[j][:, s0:s0 + 128],
                              strip[:, 2 * CH - 256 + 128 * j:2 * CH - 256 + 128 * (j + 1)],
                              transpose=True)

    # ================= Phase B: conv branch =================
    stackA.close()
    stackB = ExitStack()
    cv_ps = stackB.enter_context(tc.tile_pool(name="cv_ps", bufs=2, space="PSUM"))
    gs_po = stackB.enter_context(tc.tile_pool(name="gs", bufs=1))
    cxf = gs_po.tile([128, SV], F32, name="cxf")   # conv1 pre-activation, f32
    for f in range(F):
        shifts = [d for d in (-1, 0, 1) if 0 <= f + d < F]
        cg = cv_ps.tile([128, HW2], F32, name="cg")
        n = len(shifts) * NKC
        i = 0
        for d in shifts:
            cs = T + HW2 * (f + d)
            for kt in range(NKC):
                nc.tensor.matmul(cg[:, :],
                                 w1_t[kt][:, RANK * (d + 1):RANK * (d + 2)],
                                 hsT_t[kt][:, cs:cs + HW2],
                                 start=(i == 0), stop=(i == n - 1))
                i += 1
        nc.scalar.activation(cxf[:, HW2 * f:HW2 * (f + 1)], cg[:, :], AT.Copy)
    gmid = gs_po.tile([128, SV], BF16, name="gmid")
    # exact gelu(x) = x * 0.5*(1 + erf(x/sqrt(2))), erf via Abramowitz-Stegun
    # 7.1.26 (|err| <= 1.5e-7), built only from sim+hw-supported primitives.
    A1, A2, A3, A4, A5 = (0.254829592, -0.284496736, 1.421413741,
                          -1.453152027, 1.061405429)
    CC = 0.3275911
    ISQ2 = float(1.0 / np.sqrt(2.0))
    ge = gs_po.tile([128, SV], F32, name="ge")     # exp(-x^2/2)
    nc.scalar.activation(ge[:, :], cxf[:, :], AT.Square, scale=ISQ2)
    nc.scalar.activation(ge[:, :], ge[:, :], AT.Exp, scale=-1.0)
    gt = gs_po.tile([128, SV], F32, name="gt")     # t = 1/(1 + c*|x|/sqrt2)
    nc.scalar.activation(gt[:, :], cxf[:, :], AT.Abs, scale=ISQ2)
    nc.vector.tensor_scalar(gt[:, :], gt[:, :], CC, 1.0, OP.mult, OP.add)
    nc.vector.reciprocal(gt[:, :], gt[:, :])
    gp = gs_po.tile([128, SV], F32, name="gp")     # Horner poly in t
    nc.vector.tensor_scalar(gp[:, :], gt[:, :], A5, A4, OP.mult, OP.add)
    for ak in (A3, A2, A1):
        nc.vector.tensor_tensor(gp[:, :], gp[:, :], gt[:, :], OP.mult)
        nc.vector.tensor_scalar_add(gp[:, :], gp[:, :], ak)
    nc.vector.tensor_tensor(gp[:, :], gp[:, :], gt[:, :], OP.mult)
    # erf(|z|) = 1 - p*e ; phi = 0.5*(1 + sign(x)*(1 - p*e))
    nc.vector.tensor_tensor(gp[:, :], gp[:, :], ge[:, :], OP.mult)
    nc.vector.tensor_scalar(gp[:, :], gp[:, :], -1.0, 1.0, OP.mult, OP.add)
    nc.scalar.activation(ge[:, :], cxf[:, :], AT.Sign)
    nc.vector.tensor_tensor(gp[:, :], gp[:, :], ge[:, :], OP.mult)
    nc.vector.tensor_scalar(gp[:, :], gp[:, :], 0.5, 0.5, OP.mult, OP.add)
    nc.vector.tensor_tensor(gmid[:, :], gp[:, :], cxf[:, :], OP.mult)
    for f in range(F):
        shifts = [d for d in (-1, 0, 1) if 0 <= f + d < F]
        for m in range(2):
            c2 = cv_ps.tile([128, HW2], F32, name="c2")
            for i, d in enumerate(shifts):
                nc.tensor.matmul(c2[:, :],
                                 w2_t[:, CH * (d + 1) + 128 * m:CH * (d + 1) + 128 * (m + 1)],
                                 gmid[:, HW2 * (f + d):HW2 * (f + d + 1)],
                                 start=(i == 0), stop=(i == len(shifts) - 1))
            nc.scalar.activation(convT[m][:, HW2 * f:HW2 * (f + 1)], c2[:, :],
                                 AT.Copy)

    # ================= Phase C: attention + out-proj + ReduceScatter =========
    stackB.close()
    sc_ps = stack.enter_context(tc.tile_pool(name="sc_ps", bufs=2, space="PSUM"))
    pv_ps = stack.enter_context(tc.tile_pool(name="pv_ps", bufs=1, space="PSUM"))
    op_ps = stack.enter_context(tc.tile_pool(name="op_ps", bufs=2, space="PSUM"))
    pt_po = pool("pt", 4)
    rec_po = pool("rec", 2)
    recb_po = pool("recb", 2)
    y_po = pool("y", 3)
    o_po = pool("o", 2)
    q0 = 0
    for qt, TQ in enumerate(QT_SIZES):
        # heads processed in pairs: the two K=64 score matmuls sit on
        # different PE row groups (base partition 0 vs 64) and overlap.
        for hp in range(2):
            pvs = [pv_ps.tile([65, 512], F32, name="pva"),
                   pv_ps.tile([65, 512], F32, name="pvb")]
            for kt in range(NST):
                pts = []
                for o in range(2):
                    ob = 64 * o
                    sc = sc_ps.tile([128, 512], F32,
                                    name="sca" if o == 0 else "scb")
                    nc.tensor.matmul(sc[:, 0:TQ],
                                     KT[hp][ob:ob + 64, 128 * kt:128 * (kt + 1)],
                                     QT[hp][ob:ob + 64, q0:q0 + TQ],
                                     start=True, stop=True)
                    pt = pt_po.tile([128, 512], BF16,
                                    name="pta" if o == 0 else "ptb")
                    nc.scalar.activation(pt[:, 0:TQ], sc[:, 0:TQ], AT.Exp,
                                         scale=0.125)
                    pts.append(pt)
                for o in range(2):
                    h = 2 * hp + o
                    nc.tensor.matmul(pvs[o][:, 0:TQ],
                                     Vt[kt][:, 65 * h:65 * h + 65],
                                     pts[o][:, 0:TQ],
                                     start=(kt == 0), stop=(kt == NST - 1))
            for o in range(2):
                ob = 64 * o
                pv = pvs[o]
                rec = rec_po.tile([1, 512], F32, name="rec")
                nc.vector.reciprocal(rec[:, 0:TQ], pv[64:65, 0:TQ])
                recb = recb_po.tile([64, 512], F32, name="recb")
                nc.gpsimd.partition_broadcast(recb[:, 0:TQ], rec[:, 0:TQ])
                nc.vector.tensor_tensor(attnT[hp][ob:ob + 64, q0:q0 + TQ],
                                        pv[0:64, 0:TQ], recb[:, 0:TQ], OP.mult)
        # add conv branch into video columns of this q-tile
        a = max(q0, T)
        b = min(q0 + TQ, S)
        if a < b:
            for m in range(2):
                nc.vector.tensor_tensor(attnT[m][:, a:b], attnT[m][:, a:b],
                                        convT[m][:, a - T:b - T], OP.add)
        # out-proj partial for this q-tile -> DRAM bounce (bf16)
        ycc_in = dram.tile([DIM, TQ], BF16, name=f"ycc_in_{qt}", tag=f"ycc_in_{qt}")
        ycc_out = dram.tile([OROWS, TQ], BF16, name=f"ycc_out_{qt}", tag=f"ycc_out_{qt}")
        for m in range(15):
            op = op_ps.tile([128, 512], F32, name="op")
            nc.tensor.matmul(op[:, 0:TQ], wo_t[0][:, 128 * m:128 * (m + 1)],
                             attnT[0][:, q0:q0 + TQ], start=True, stop=False)
            nc.tensor.matmul(op[:, 0:TQ], wo_t[1][:, 128 * m:128 * (m + 1)],
                             attnT[1][:, q0:q0 + TQ], start=False, stop=True)
            ysb = y_po.tile([128, 512], BF16, name="ysb")
            nc.vector.tensor_copy(ysb[:, 0:TQ], op[:, 0:TQ])
            nc.sync.dma_start(ycc_in[128 * m:128 * (m + 1), :], ysb[:, 0:TQ])
        nc.gpsimd.collective_compute(
            "ReduceScatter", mybir.AluOpType.add,
            replica_groups=[list(range(NCORE))],
            ins=[ycc_in[:, :].opt()],
            outs=[ycc_out[:, :].opt()],
        )
        # bias + f32 cast + final output DMA
        for pb, pn in ((0, 128), (128, OROWS - 128)):
            ot = o_po.tile([128, 512], BF16, name="ot")
            nc.sync.dma_start(ot[0:pn, 0:TQ], ycc_out[pb:pb + pn, :])
            of = o_po.tile([128, 512], F32, name="of")
            nc.scalar.activation(of[0:pn, 0:TQ], ot[0:pn, 0:TQ], AT.Identity,
                                 bias=bout_t[0:pn, (0 if pb == 0 else 1):(1 if pb == 0 else 2)])
            nc.sync.dma_start(y_d[pb:pb + pn, q0:q0 + TQ], of[0:pn, 0:TQ])
        q0 += TQ

    stack.close()


_NC = None


def _get_nc():
    global _NC
    if _NC is None:
        _NC = build_nc()
    return _NC


def kernel(**inputs):
    nc = _get_nc()
    in_maps = _host_prep(inputs)
    res = bass_utils.run_bass_kernel_spmd(nc, in_maps,
                                          core_ids=list(range(NCORE)))
    yT = np.concatenate([np.asarray(res.results[i]["yslice"])
                         for i in range(NCORE)], 0)  # [1920, SP]
    out = yT[:, :S].T.astype(np.float32)             # [S, DIM]
    return out[None, T:], out[None, :T]


# revision 16
# speedup vs baseline: 1.1570x; 1.1570x over previous
"""Trainium2 8-core Bass kernel for CogVideoX attention + conv1d block.

Sharding: heads (tensor-parallel, 30 padded to 32 -> 4 heads/core) for
QKV/attention; out-proj computed as per-core channel partials combined with a
chunked ReduceScatter along the sequence; conv1d branch: conv1 replicated,
conv2 sharded by output channel so it folds into the same partial.

All matmuls in bf16 (fp32 PSUM accumulation). Interleaved RoPE is turned into
a rotate-half form by permuting Q/K weight rows host-side. QKV bias is folded
into the projection via an ones-row in hsT. Softmax denominators come from an
ones-column appended to V. LayerNorm q/k weights are identity (per the module
spec) so only (x-mu)*rsqrt(var+eps) is applied.

Schedule: the conv branch is emitted first so its matmuls fill the input-DMA
ramp; attention runs as score-bursts then PV-bursts over kt-paired two-bank
PSUM tiles (one exp instruction per kt pair) to keep the PE dense.
"""
import numpy as np
import ml_dtypes

from concourse import bacc, tile, mybir
from concourse import bass_utils

BF16 = mybir.dt.bfloat16
F32 = mybir.dt.float32
BF = ml_dtypes.bfloat16

B, F, H, W = 1, 8, 16, 16
DIM, HEADS, HD, RANK, T = 1920, 30, 64, 128, 226
SV = F * H * W            # 2048
S = T + SV                # 2274
SP = 2304                 # padded S = 18*128
NST = SP // 128           # 18 s-tiles
DP = 2048                 # padded contraction dim (1920 + ones row + zeros)
NKT = DP // 128           # 16 k-tiles
NKC = DIM // 128          # 15 k-tiles for conv (no ones row)
HP = 32                   # padded heads
NCORE = 8
HPC = HP // NCORE         # 4 heads/core
CH = HPC * HD             # 256 channels/core
VCH = HPC * (HD + 1)      # 260 v channels (with ones cols)
OROWS = DIM // NCORE      # 240 output rows/core after ReduceScatter
EPS = 1e-6
QT_SIZES = [512, 512, 512, 512, 256]  # 5 q-tiles covering SP
HW2 = H * W               # 256

PERM = np.concatenate([np.arange(0, 64, 2), np.arange(1, 64, 2)])


def _host_prep(inputs):
    hs = np.asarray(inputs["hidden_states"], np.float32)[0]
    ehs = np.asarray(inputs["encoder_hidden_states"], np.float32)[0]
    cos = np.asarray(inputs["cos"], np.float32)
    sin = np.asarray(inputs["sin"], np.float32)

    hs_cat = np.concatenate([ehs, hs], 0)
    hsT = np.zeros((DP, SP), np.float32)
    hsT[:DIM, :S] = hs_cat.T
    hsT[DIM, :S] = 1.0

    def pad_rows(w, b, deint):
        wp = np.zeros((HP * HD, DIM), np.float32)
        bp = np.zeros((HP * HD,), np.float32)
        wp[:DIM] = np.asarray(w, np.float32)
        bp[:DIM] = np.asarray(b, np.float32)
        if deint:
            wp = wp.reshape(HP, HD, DIM)[:, PERM].reshape(HP * HD, DIM)
            bp = bp.reshape(HP, HD)[:, PERM].reshape(-1)
        return wp, bp

    wq, bq = pad_rows(inputs["to_q_w"], inputs["to_q_b"], True)
    wk, bk = pad_rows(inputs["to_k_w"], inputs["to_k_b"], True)
    wv, bv = pad_rows(inputs["to_v_w"], inputs["to_v_b"], False)

    cosD = cos[:, PERM]
    sinF = sin[:, PERM].copy()
    sinF[:, :32] *= -1.0
    cosP = np.ones((SP, CH), np.float32)
    sinP = np.zeros((SP, CH), np.float32)
    cosP[T:S] = np.tile(cosD, HPC)
    sinP[T:S] = np.tile(sinF, HPC)

    woT_full = np.zeros((HP * HD, DIM), np.float32)
    woT_full[:DIM] = np.asarray(inputs["to_out_w"], np.float32).T

    w1 = np.asarray(inputs["conv1_w"], np.float32)
    w2 = np.asarray(inputs["conv2_w"], np.float32)
    w1T = np.zeros((DIM, 3 * RANK), np.float32)
    for k in range(3):
        w1T[:, RANK * k:RANK * (k + 1)] = w1[:, :, k].T
    bout = np.asarray(inputs["to_out_b"], np.float32)

    in_maps = []
    for i in range(NCORE):
        hsl = slice(CH * i, CH * (i + 1))
        wqkT = np.zeros((DP, 2 * CH), np.float32)
        wqkT[:DIM, :CH] = wq[hsl].T
        wqkT[DIM, :CH] = bq[hsl]
        wqkT[:DIM, CH:] = wk[hsl].T
        wqkT[DIM, CH:] = bk[hsl]

        wvT = np.zeros((DP, VCH), np.float32)
        for h in range(HPC):
            c0 = 65 * h
            rs = slice(CH * i + HD * h, CH * i + HD * (h + 1))
            wvT[:DIM, c0:c0 + HD] = wv[rs].T
            wvT[DIM, c0:c0 + HD] = bv[rs]
            wvT[DIM, c0 + HD] = 1.0

        w2T = np.zeros((RANK, 3 * CH), np.float32)
        lo, hi = CH * i, min(CH * (i + 1), DIM)
        for k in range(3):
            blk = np.zeros((CH, RANK), np.float32)
            if lo < DIM:
                blk[:hi - lo] = w2[lo:hi, :, k]
            w2T[:, CH * k:CH * (k + 1)] = blk.T

        bslice = bout[OROWS * i:OROWS * (i + 1)].reshape(OROWS, 1).copy()

        in_maps.append({
            "hsT": hsT.astype(BF),
            "wqkT": wqkT.astype(BF),
            "wvT": wvT.astype(BF),
            "woT": woT_full[hsl].astype(BF),
            "w1T": w1T.astype(BF),
            "w2T": w2T.astype(BF),
            "cosP": cosP.astype(BF),
            "sinP": sinP.astype(BF),
            "bout": bslice.astype(np.float32),
        })
    return in_maps


def build_nc(repeat=1):
    nc = bacc.Bacc("TRN2", target_bir_lowering=False, debug=False,
                   num_devices=NCORE)

    hsT_d = nc.dram_tensor("hsT", [DP, SP], BF16, kind="ExternalInput").ap()
    wqkT_d = nc.dram_tensor("wqkT", [DP, 2 * CH], BF16, kind="ExternalInput").ap()
    wvT_d = nc.dram_tensor("wvT", [DP, VCH], BF16, kind="ExternalInput").ap()
    woT_d = nc.dram_tensor("woT", [CH, DIM], BF16, kind="ExternalInput").ap()
    w1T_d = nc.dram_tensor("w1T", [DIM, 3 * RANK], BF16, kind="ExternalInput").ap()
    w2T_d = nc.dram_tensor("w2T", [RANK, 3 * CH], BF16, kind="ExternalInput").ap()
    cosP_d = nc.dram_tensor("cosP", [SP, CH], BF16, kind="ExternalInput").ap()
    sinP_d = nc.dram_tensor("sinP", [SP, CH], BF16, kind="ExternalInput").ap()
    bout_d = nc.dram_tensor("bout", [OROWS, 1], F32, kind="ExternalInput").ap()
    y_d = nc.dram_tensor("yslice", [OROWS, SP], F32, kind="ExternalOutput").ap()

    with tile.TileContext(nc) as tc:
        for _ in range(repeat):
            _body(nc, tc, hsT_d, wqkT_d, wvT_d, woT_d, w1T_d, w2T_d,
                  cosP_d, sinP_d, bout_d, y_d)
    nc.compile()
    return nc


def _body(nc, tc, hsT_d, wqkT_d, wvT_d, woT_d, w1T_d, w2T_d,
          cosP_d, sinP_d, bout_d, y_d):
    from contextlib import ExitStack
    stack = ExitStack()

    def pool(name, bufs, space="SBUF"):
        return stack.enter_context(tc.tile_pool(name=name, bufs=bufs, space=space))

    const = pool("const", 1)
    dram = pool("dram", 1, space="DRAM")

    stackA = ExitStack()
    aw_po = stackA.enter_context(tc.tile_pool(name="aw", bufs=1))

    # ---- input DMAs: hsT + conv weights first (the conv branch is the first
    # PE work and needs only these), then QKV weights, then the rest ----
    hsT_t, w1_t = [], []
    for k in range(NKT):
        t = const.tile([128, SP], BF16, name=f"hsT_{k}")
        nc.sync.dma_start(t[:, :], hsT_d[128 * k:128 * (k + 1), :])
        hsT_t.append(t)
        if k < NKC:
            t = const.tile([128, 3 * RANK], BF16, name=f"w1_{k}")
            nc.sync.dma_start(t[:, :], w1T_d[128 * k:128 * (k + 1), :])
            w1_t.append(t)
    w2_t = const.tile([128, 3 * CH], BF16, name="w2")
    nc.sync.dma_start(w2_t[:, :], w2T_d[:, :])
    wqk_t, wv_t = [], []
    for k in range(NKT):
        t = aw_po.tile([128, 2 * CH], BF16, name=f"wqk_{k}")
        nc.sync.dma_start(t[:, :], wqkT_d[128 * k:128 * (k + 1), :])
        wqk_t.append(t)
        t = aw_po.tile([128, VCH], BF16, name=f"wv_{k}")
        nc.sync.dma_start(t[:, :], wvT_d[128 * k:128 * (k + 1), :])
        wv_t.append(t)
    wo_t = []
    for m in range(2):
        t = const.tile([128, DIM], BF16, name=f"wo_{m}")
        nc.sync.dma_start(t[:, :], woT_d[128 * m:128 * (m + 1), :])
        wo_t.append(t)
    eps_t = const.tile([128, 1], F32, name="eps")
    nc.vector.memset(eps_t[:, :], EPS)
    bout_t = const.tile([128, 2], F32, name="bout")
    nc.sync.dma_start(bout_t[0:128, 0:1], bout_d[0:128, :])
    nc.sync.dma_start(bout_t[0:OROWS - 128, 1:2], bout_d[128:OROWS, :])

    QT = [const.tile([128, SP], BF16, name=f"QT_{j}") for j in range(2)]
    KT = [const.tile([128, SP], BF16, name=f"KT_{j}") for j in range(2)]
    Vt = [const.tile([128, VCH], BF16, name=f"V_{st}") for st in range(NST)]
    attnT = [const.tile([128, SP], BF16, name=f"attnT_{m}") for m in range(2)]
    convT = [const.tile([128, SV], BF16, name=f"convT_{m}") for m in range(2)]
    gmid = const.tile([128, SV], BF16, name="gmid")

    # phase A pools (PSUM: qk 2 + v 2 + cv 2 = 6 of 8 banks)
    qk_ps = stackA.enter_context(tc.tile_pool(name="qk_ps", bufs=2, space="PSUM"))
    v_ps = stackA.enter_context(tc.tile_pool(name="v_ps", bufs=2, space="PSUM"))
    cv_ps = stackA.enter_context(tc.tile_pool(name="cv_ps", bufs=2, space="PSUM"))
    sq_po = stackA.enter_context(tc.tile_pool(name="sq", bufs=2))
    st_po = stackA.enter_context(tc.tile_pool(name="stats", bufs=3))
    strip_po = stackA.enter_context(tc.tile_pool(name="strip", bufs=3))
    rope_po = stackA.enter_context(tc.tile_pool(name="rope", bufs=2))
    cos_po = stackA.enter_context(tc.tile_pool(name="cos", bufs=2))
    gs_po = stackA.enter_context(tc.tile_pool(name="gs", bufs=1))

    AT = mybir.ActivationFunctionType
    OP = mybir.AluOpType
    AX = mybir.AxisListType

    # ================= conv branch (emitted first: fills the DMA ramp) ======
    # conv1 + exact gelu in two halves (bf16 scratch), then conv2.
    A1, A2, A3, A4, A5 = (0.254829592, -0.284496736, 1.421413741,
                          -1.453152027, 1.061405429)
    CC = 0.3275911
    ISQ2 = float(1.0 / np.sqrt(2.0))
    HSV = SV // 2
    lp = nc.allow_low_precision(reason="elementwise gelu chain in bf16; no accumulation")
    lp.__enter__()
    for half in range(2):
        xh = gs_po.tile([128, HSV], BF16, name="xh")
        for fi in range(4):
            f = 4 * half + fi
            shifts = [d for d in (-1, 0, 1) if 0 <= f + d < F]
            cg = cv_ps.tile([128, HW2], F32, name="cg")
            n = len(shifts) * NKC
            i = 0
            for d in shifts:
                cs = T + HW2 * (f + d)
                for kt in range(NKC):
                    nc.tensor.matmul(cg[:, :],
                                     w1_t[kt][:, RANK * (d + 1):RANK * (d + 2)],
                                     hsT_t[kt][:, cs:cs + HW2],
                                     start=(i == 0), stop=(i == n - 1))
                    i += 1
            nc.vector.tensor_copy(xh[:, HW2 * fi:HW2 * (fi + 1)], cg[:, :])
        # gelu(x) = x * 0.5*(1+erf(x/sqrt2)); erf via Abramowitz-Stegun 7.1.26
        ge = gs_po.tile([128, HSV], BF16, name="ge")
        nc.scalar.activation(ge[:, :], xh[:, :], AT.Square, scale=ISQ2)
        nc.scalar.activation(ge[:, :], ge[:, :], AT.Exp, scale=-1.0)
        gt = gs_po.tile([128, HSV], BF16, name="gt")
        nc.scalar.activation(gt[:, :], xh[:, :], AT.Abs, scale=ISQ2)
        nc.vector.tensor_scalar(gt[:, :], gt[:, :], CC, 1.0, OP.mult, OP.add)
        nc.vector.reciprocal(gt[:, :], gt[:, :])
        gp = gs_po.tile([128, HSV], BF16, name="gp")
        nc.vector.tensor_scalar(gp[:, :], gt[:, :], A5, A4, OP.mult, OP.add)
        for ak in (A3, A2, A1):
            nc.vector.tensor_tensor(gp[:, :], gp[:, :], gt[:, :], OP.mult)
            nc.vector.tensor_scalar_add(gp[:, :], gp[:, :], ak)
        nc.vector.tensor_tensor(gp[:, :], gp[:, :], gt[:, :], OP.mult)
        nc.vector.tensor_tensor(gp[:, :], gp[:, :], ge[:, :], OP.mult)
        nc.vector.tensor_scalar(gp[:, :], gp[:, :], -1.0, 1.0, OP.mult, OP.add)
        nc.scalar.activation(ge[:, :], xh[:, :], AT.Sign)
        nc.vector.tensor_tensor(gp[:, :], gp[:, :], ge[:, :], OP.mult)
        nc.vector.tensor_scalar(gp[:, :], gp[:, :], 0.5, 0.5, OP.mult, OP.add)
        nc.vector.tensor_tensor(gmid[:, HSV * half:HSV * (half + 1)],
                                gp[:, :], xh[:, :], OP.mult)
    lp.__exit__(None, None, None)
    for f in range(F):
        shifts = [d for d in (-1, 0, 1) if 0 <= f + d < F]
        for m in range(2):
            c2 = cv_ps.tile([128, HW2], F32, name="c2")
            for i, d in enumerate(shifts):
                nc.tensor.matmul(c2[:, :],
                                 w2_t[:, CH * (d + 1) + 128 * m:CH * (d + 1) + 128 * (m + 1)],
                                 gmid[:, HW2 * (f + d):HW2 * (f + d + 1)],
                                 start=(i == 0), stop=(i == len(shifts) - 1))
            nc.vector.tensor_copy(convT[m][:, HW2 * f:HW2 * (f + 1)], c2[:, :])

    # ================= Phase A: QKV + LN + RoPE + transposes =================
    for st in range(NST):
        s0 = 128 * st
        qk_p = qk_ps.tile([128, 2 * CH], F32, name="qk_p")
        v_p = v_ps.tile([128, VCH], F32, name="v_p")
        for kt in range(NKT):
            lhsT = hsT_t[kt][:, s0:s0 + 128]
            nc.tensor.matmul(qk_p[:, :], lhsT, wqk_t[kt][:, :],
                             start=(kt == 0), stop=(kt == NKT - 1))
            nc.tensor.matmul(v_p[:, :], lhsT, wv_t[kt][:, :],
                             start=(kt == 0), stop=(kt == NKT - 1))
        # V straight to bf16 (bias already folded via ones row)
        nc.vector.tensor_copy(Vt[st][:, :], v_p[:, :])

        # layernorm stats per 64-col group (8 groups: 4 q heads + 4 k heads)
        sq = sq_po.tile([128, 2 * CH], F32, name="sq")
        nc.scalar.activation(sq[:, :], qk_p[:, :], AT.Square)
        sums = st_po.tile([128, 8], F32, name="sums")
        sqs = st_po.tile([128, 8], F32, name="sqs")
        nc.vector.tensor_reduce(sums[:, :],
                                qk_p[:, :].rearrange("p (g d) -> p g d", g=8),
                                AX.X, OP.add)
        nc.vector.tensor_reduce(sqs[:, :],
                                sq[:, :].rearrange("p (g d) -> p g d", g=8),
                                AX.X, OP.add)
        mu = st_po.tile([128, 8], F32, name="mu")
        nc.vector.tensor_scalar_mul(mu[:, :], sums[:, :], 1.0 / HD)
        msq = st_po.tile([128, 8], F32, name="msq")
        nc.vector.tensor_scalar_mul(msq[:, :], sqs[:, :], 1.0 / HD)
        mu2 = st_po.tile([128, 8], F32, name="mu2")
        nc.vector.tensor_tensor(mu2[:, :], mu[:, :], mu[:, :], OP.mult)
        var = st_po.tile([128, 8], F32, name="var")
        nc.vector.tensor_tensor(var[:, :], msq[:, :], mu2[:, :], OP.subtract)
        std = st_po.tile([128, 8], F32, name="std")
        nc.scalar.activation(std[:, :], var[:, :], AT.Sqrt, bias=eps_t[:, 0:1])
        rstd = st_po.tile([128, 8], F32, name="rstd")
        nc.vector.reciprocal(rstd[:, :], std[:, :])

        strip = strip_po.tile([128, 2 * CH], BF16, name="strip")
        for g in range(8):
            nc.vector.tensor_scalar(strip[:, HD * g:HD * (g + 1)],
                                    qk_p[:, HD * g:HD * (g + 1)],
                                    mu[:, g:g + 1], rstd[:, g:g + 1],
                                    OP.subtract, OP.mult)

        # rope on q half and k half (identity rows cover text/pad)
        cos_t = cos_po.tile([128, CH], BF16, name="cos_t")
        sin_t = cos_po.tile([128, CH], BF16, name="sin_t")
        nc.sync.dma_start(cos_t[:, :], cosP_d[s0:s0 + 128, :])
        nc.sync.dma_start(sin_t[:, :], sinP_d[s0:s0 + 128, :])
        for part in range(2):
            c0 = CH * part
            x3 = strip[:, c0:c0 + CH].rearrange("p (h d) -> p h d", h=HPC)
            rot = rope_po.tile([128, CH], BF16, name="rot")
            rot3 = rot[:, :].rearrange("p (h d) -> p h d", h=HPC)
            sin3 = sin_t[:, :].rearrange("p (h d) -> p h d", h=HPC)
            nc.vector.tensor_tensor(rot3[:, :, 0:32], x3[:, :, 32:64],
                                    sin3[:, :, 0:32], OP.mult)
            nc.vector.tensor_tensor(rot3[:, :, 32:64], x3[:, :, 0:32],
                                    sin3[:, :, 32:64], OP.mult)
            xc = rope_po.tile([128, CH], BF16, name="xc")
            nc.vector.tensor_tensor(xc[:, :], strip[:, c0:c0 + CH],
                                    cos_t[:, :], OP.mult)
            nc.vector.tensor_tensor(strip[:, c0:c0 + CH], xc[:, :],
                                    rot[:, :], OP.add)

        # transpose into QT/KT via DMA transpose (bf16, 128x128 blocks)
        for j in range(2):
            nc.sync.dma_start(QT[j][:, s0:s0 + 128],
                              strip[:, 128 * j:128 * (j + 1)], transpose=True)
            nc.sync.dma_start(KT[j][:, s0:s0 + 128],
                              strip[:, CH + 128 * j:CH + 128 * (j + 1)],
                              transpose=True)

    # ================= Phase C: attention + out-proj + ReduceScatter =========
    stackA.close()
    sc_ps = stack.enter_context(tc.tile_pool(name="sc_ps", bufs=2, space="PSUM"))
    pv_ps = stack.enter_context(tc.tile_pool(name="pv_ps", bufs=1, space="PSUM"))
    op_ps = stack.enter_context(tc.tile_pool(name="op_ps", bufs=2, space="PSUM"))
    pt_po = pool("pt", 5)
    rec_po = pool("rec", 2)
    recb_po = pool("recb", 2)
    y_po = pool("y", 3)
    o_po = pool("o", 2)

    NPAIR = NST // 2          # 9 kt-pairs
    q0 = 0
    for qt, TQ in enumerate(QT_SIZES):
        for hp in range(2):
            pvs = [pv_ps.tile([65, 512], F32, name="pva"),
                   pv_ps.tile([65, 512], F32, name="pvb")]
            for burst in (range(0, 5), range(5, NPAIR)):
                pts = {}
                # scores burst: kt-paired two-bank PSUM tiles, one exp per pair
                for p in burst:
                    for o in range(2):
                        ob = 64 * o
                        h = 2 * hp + o
                        sc = sc_ps.tile([128, 1024], F32, name="sc")
                        for s_i in range(2):
                            kt = 2 * p + s_i
                            nc.tensor.matmul(
                                sc[:, 512 * s_i:512 * s_i + TQ],
                                KT[hp][ob:ob + 64, 128 * kt:128 * (kt + 1)],
                                QT[hp][ob:ob + 64, q0:q0 + TQ],
                                start=True, stop=True)
                        pt = pt_po.tile([128, 1024], BF16,
                                        name="pta" if o == 0 else "ptb")
                        sc3 = sc[:, :].rearrange("p (a b) -> p a b", a=2)
                        pt3 = pt[:, :].rearrange("p (a b) -> p a b", a=2)
                        nc.scalar.activation(pt3[:, :, 0:TQ], sc3[:, :, 0:TQ],
                                             AT.Exp, scale=0.125)
                        pts[(p, o)] = pt
                # PV burst
                for p in burst:
                    for o in range(2):
                        h = 2 * hp + o
                        for s_i in range(2):
                            kt = 2 * p + s_i
                            nc.tensor.matmul(
                                pvs[o][:, 0:TQ],
                                Vt[kt][:, 65 * h:65 * h + 65],
                                pts[(p, o)][:, 512 * s_i:512 * s_i + TQ],
                                start=(kt == 0), stop=(kt == NST - 1))
            for o in range(2):
                ob = 64 * o
                pv = pvs[o]
                rec = rec_po.tile([1, 512], F32, name="rec")
                nc.vector.reciprocal(rec[:, 0:TQ], pv[64:65, 0:TQ])
                recb = recb_po.tile([64, 512], F32, name="recb")
                nc.gpsimd.partition_broadcast(recb[:, 0:TQ], rec[:, 0:TQ])
                nc.vector.tensor_tensor(attnT[hp][ob:ob + 64, q0:q0 + TQ],
                                        pv[0:64, 0:TQ], recb[:, 0:TQ], OP.mult)
        # add conv branch into video columns of this q-tile
        a = max(q0, T)
        b = min(q0 + TQ, S)
        if a < b:
            for m in range(2):
                nc.vector.tensor_tensor(attnT[m][:, a:b], attnT[m][:, a:b],
                                        convT[m][:, a - T:b - T], OP.add)
        # out-proj partial for this q-tile -> DRAM bounce (bf16)
        ycc_in = dram.tile([DIM, TQ], BF16, name=f"ycc_in_{qt}", tag=f"ycc_in_{qt}")
        ycc_out = dram.tile([OROWS, TQ], BF16, name=f"ycc_out_{qt}", tag=f"ycc_out_{qt}")
        for m in range(15):
            op = op_ps.tile([128, 512], F32, name="op")
            nc.tensor.matmul(op[:, 0:TQ], wo_t[0][:, 128 * m:128 * (m + 1)],
                             attnT[0][:, q0:q0 + TQ], start=True, stop=False)
            nc.tensor.matmul(op[:, 0:TQ], wo_t[1][:, 128 * m:128 * (m + 1)],
                             attnT[1][:, q0:q0 + TQ], start=False, stop=True)
            ysb = y_po.tile([128, 512], BF16, name="ysb")
            nc.vector.tensor_copy(ysb[:, 0:TQ], op[:, 0:TQ])
            nc.sync.dma_start(ycc_in[128 * m:128 * (m + 1), :], ysb[:, 0:TQ])
        nc.gpsimd.collective_compute(
            "ReduceScatter", mybir.AluOpType.add,
            replica_groups=[list(range(NCORE))],
            ins=[ycc_in[:, :].opt()],
            outs=[ycc_out[:, :].opt()],
        )
        # bias + f32 cast + final output DMA
        for pb, pn in ((0, 128), (128, OROWS - 128)):
            ot = o_po.tile([128, 512], BF16, name="ot")
            nc.sync.dma_start(ot[0:pn, 0:TQ], ycc_out[pb:pb + pn, :])
            of = o_po.tile([128, 512], F32, name="of")
            nc.scalar.activation(of[0:pn, 0:TQ], ot[0:pn, 0:TQ], AT.Identity,
                                 bias=bout_t[0:pn, (0 if pb == 0 else 1):(1 if pb == 0 else 2)])
            nc.sync.dma_start(y_d[pb:pb + pn, q0:q0 + TQ], of[0:pn, 0:TQ])
        q0 += TQ

    stack.close()


_NC = None


def _get_nc():
    global _NC
    if _NC is None:
        _NC = build_nc()
    return _NC


def kernel(**inputs):
    nc = _get_nc()
    in_maps = _host_prep(inputs)
    res = bass_utils.run_bass_kernel_spmd(nc, in_maps,
                                          core_ids=list(range(NCORE)))
    yT = np.concatenate([np.asarray(res.results[i]["yslice"])
                         for i in range(NCORE)], 0)  # [1920, SP]
    out = yT[:, :S].T.astype(np.float32)             # [S, DIM]
    return out[None, T:], out[None, :T]
